# revision 50
# baseline (speedup 1.0000x reference)
"""GAT layer (PyG-style, add_self_loops=True) on 8 Trainium2 NeuronCores.

Strategy: partition destination nodes (and their incident edges) across the 8
cores; each core owns a contiguous range of 6250 dst nodes (49 windows of 128).

No projection table. Per window of 128 dst nodes, two transposed dma_gathers
(lo/hi halves of the node range, int16-index limit) pull the raw 256-byte x
rows of all incident edges' sources straight out of HBM, TRANSPOSED at u16
granularity: the host pre-interleaves each x row's bytes as
[hi16(x_0)..hi16(x_63) | lo16(x_0)..lo16(x_63)], so gather partitions 0:64
hold truncated-bf16 features and serve directly as the matmul lhsT. Each
128-edge subtile then computes h|a_src = x_src @ [W | W@att_src^T] as ONE bf16
matmul (f32 PSUM), so there is no replicated 50k-row projection pass and no
38 MB table write at all.

The edge->slot one-hot matrices are pure index data, so the HOST builds them
(both layouts: edge-major for the segment-sum lhsT, slot-major for the a_dst
expansion) as bf16 bit patterns, streamed in per window by DMA -- no DVE
one-hot build, no PE transposes, no PSUM->SBUF copies. a_src accumulates with
a_dst (ohT_w @ a_dst_window matmul) into one tiny score PSUM via start/stop
chaining, with 0.2-scaled twin weight columns so the score psum holds
[s | 0.2*s] and leaky_relu is a single DVE reduce_max over the pair; Act exp
writes the e-columns of the msg tile directly; DVE fuses the PSUM->SBUF h move
with the e-scale
(msg = h*e, bf16). Segment-sum of [e*h | e] via PSUM-accumulated one-hot
matmuls, software-pipelined so projection matmuls run a chunk ahead of the
accumulation matmuls; final out = acc/(denom+eps) + bias, batched 7 windows
per output DMA. Softmax max-subtraction is skipped (shift-invariant; scores
are O(1) so exp cannot overflow).

Pad edges gather row 0 (finite) and carry dst-slot sentinel 500 => their
one-hot row is all zero, so they contribute to nothing. No dummy rows.

All small per-window inputs (gather indices, one-hots) are laid out
partition-major on the host so every DMA is a few large contiguous
descriptors -- per-DMA fixed cost (~2us DGE+sem) dominated the v1 kernel.

Host does only index/byte-space work (self-loop append, dst sort, windowing,
padding, int16 index wrapping, u16 byte interleave of x, one-hot bit
patterns, x transpose/slice).

Act-engine Lrelu was tried for the leaky_relu and produces WRONG results on
HW (semantics differ from alpha*min(x,0)+max(x,0)); keep the DVE max-pair.
"""

import math

import numpy as np

N = 50000
IN_DIM = 64
H = 4
D = 64
HD = H * D  # 256
WCOLS = HD + H  # 260: msg cols h | e
SC2 = 2 * H  # score psum cols per subtile: [s | 0.2*s] (leaky via reduce_max)
NEG_SLOPE = 0.2
EPS = 1e-16
SENT = 500.0  # dst-slot sentinel for pad edges (one-hot row all zero)

NCORES = 8
NPC = N // NCORES  # 6250 dst nodes per core
NWIN = math.ceil(NPC / 128)  # 49 windows
WROWS = NWIN * 128  # 6272
SPLIT = 25088  # lo/hi x-table split (int16 gather index limit)
SUBS = 4  # edge subtiles per chunk (PSUM: hps 2 banks x2, spt 1 bank x2, acc 2)
USE_ACT_LRELU = False  # Act-engine Lrelu gives WRONG results on HW; keep DVE

LAST_RESULTS = None  # BassKernelResults of the most recent run (for test.py)


def _wrap_idx(ids):
    """[n] int -> dma_gather wrapped layout [128, n/16] int16
    (idx i at [i%16, i//16], replicated across the 8 Q7 core groups)."""
    n = len(ids)
    w16 = ids.reshape(n // 16, 16).T.astype(np.int16)  # [16, n/16]
    return np.tile(w16, (8, 1))


def _interleave_x(x):
    """[N,64] f32 -> [N,128] u16 rows [hi16(x_0..63) | lo16(x_0..63)].
    After the u16-granularity transposed gather, partitions 0:64 hold the
    high halves = truncated-bf16 feature values."""
    xu = np.ascontiguousarray(x).view(np.uint16).reshape(-1, 64, 2)
    return np.ascontiguousarray(np.concatenate([xu[:, :, 1], xu[:, :, 0]], axis=1))


def _prep_host(edge_index):
    """Returns ilow  int16 [NCORES, NWIN, 128, KL*8]
               ihigh int16 [NCORES, NWIN, 128, KH*8]
               dstrel f32  [NCORES, NWIN, 128, KL+KH]  (slot or SENT)
               (KL, KH)"""
    src = np.concatenate([edge_index[0], np.arange(N, dtype=np.int64)]).astype(np.int64)
    dst = np.concatenate([edge_index[1], np.arange(N, dtype=np.int64)]).astype(np.int64)
    order = np.argsort(dst, kind="stable")
    src = src[order].astype(np.int32)
    dst = dst[order].astype(np.int32)

    bounds = [c * NPC + w * 128 for c in range(NCORES) for w in range(NWIN)]
    bounds.append(N)
    cuts = np.searchsorted(dst, np.asarray(bounds))

    lo_counts = np.zeros(NCORES * NWIN, np.int64)
    hi_counts = np.zeros(NCORES * NWIN, np.int64)
    for b in range(NCORES * NWIN):
        s = src[cuts[b] : cuts[b + 1]]
        lo_counts[b] = int((s < SPLIT).sum())
        hi_counts[b] = len(s) - lo_counts[b]
    KL = max(1, math.ceil(lo_counts.max() / 128))
    KH = max(1, math.ceil(hi_counts.max() / 128))
    kj = KL + KH

    ilow = np.zeros((NCORES, NWIN, KL * 128), np.int32)  # pad -> row 0 (finite)
    ihigh = np.zeros((NCORES, NWIN, KH * 128), np.int32)
    dstrel = np.full((NCORES, NWIN, 128, kj), SENT, np.float32)
    for c in range(NCORES):
        base = c * NPC
        for w in range(NWIN):
            b = c * NWIN + w
            s = src[cuts[b] : cuts[b + 1]]
            d = dst[cuts[b] : cuts[b + 1]] - base - w * 128
            m = s < SPLIT
            slo, dlo = s[m], d[m]
            shi, dhi = s[~m] - SPLIT, d[~m]
            # ascending source rows => HBM page locality in the gather
            o = np.argsort(slo, kind="stable")
            slo, dlo = slo[o], dlo[o]
            o = np.argsort(shi, kind="stable")
            shi, dhi = shi[o], dhi[o]
            ilow[c, w, : len(slo)] = slo
            ihigh[c, w, : len(shi)] = shi
            i = np.arange(len(slo))
            dstrel[c, w, i % 128, i // 128] = dlo
            i = np.arange(len(shi))
            dstrel[c, w, i % 128, KL + i // 128] = dhi
    ilow_w = np.zeros((NCORES, NWIN, 128, KL * 8), np.int16)
    ihigh_w = np.zeros((NCORES, NWIN, 128, KH * 8), np.int16)
    for c in range(NCORES):
        for w in range(NWIN):
            ilow_w[c, w] = _wrap_idx(ilow[c, w])
            ihigh_w[c, w] = _wrap_idx(ihigh[c, w])

    # one-hot (edge->slot) matrices in BOTH layouts as bf16 bit patterns
    # (0x3F80 = bf16 1.0), partition-major for contiguous per-window DMA:
    #   oh [p=e, w*kj*128 + s*128 + slot], ohT [p=slot, w*kj*128 + s*128 + e]
    one = np.uint16(0x3F80)
    slots = np.arange(128)
    oh_u = np.zeros((NCORES, 128, NWIN * kj * 128), np.uint16)
    ohT_u = np.zeros((NCORES, 128, NWIN * kj * 128), np.uint16)
    for c in range(NCORES):
        eq = dstrel[c][:, :, :, None] == slots  # [w, e, s, slot] bool
        oh_u[c] = (
            eq.transpose(1, 0, 2, 3).reshape(128, -1) * one
        )
        ohT_u[c] = (
            eq.transpose(3, 0, 2, 1).reshape(128, -1) * one
        )
    return ilow_w, ihigh_w, oh_u, ohT_u, KL, KH


def _build_program(KL, KH, ablate="full"):
    import concourse.bass as bass
    import concourse.bacc as bacc
    import concourse.tile as tile
    from concourse import mybir

    f32 = mybir.dt.float32
    bf16 = mybir.dt.bfloat16
    u16 = mybir.dt.uint16
    i16 = mybir.dt.int16
    kj = KL + KH

    nc = bacc.Bacc(None, target_bir_lowering=False)

    xlo_d = nc.dram_tensor("x_lo", [SPLIT, 128], u16, kind="ExternalInput")
    xhi_d = nc.dram_tensor("x_hi", [N - SPLIT, 128], u16, kind="ExternalInput")
    xdT_d = nc.dram_tensor("xdstT", [IN_DIM, WROWS], f32, kind="ExternalInput")
    W_d = nc.dram_tensor("W", [IN_DIM, HD], f32, kind="ExternalInput")
    asrc_d = nc.dram_tensor("att_src", [1, HD], f32, kind="ExternalInput")
    adst_d = nc.dram_tensor("att_dst", [1, HD], f32, kind="ExternalInput")
    bias_d = nc.dram_tensor("bias", [1, HD], f32, kind="ExternalInput")
    # partition-major on host so the one-shot preload DMA is contiguous per
    # partition (128 large descriptors, not NWIN*128 small ones)
    il_d = nc.dram_tensor("ilow", [128, NWIN * KL * 8], i16, kind="ExternalInput")
    ih_d = nc.dram_tensor("ihigh", [128, NWIN * KH * 8], i16, kind="ExternalInput")
    # host-built one-hot matrices (bf16 bit patterns), both layouts
    oh_d = nc.dram_tensor("oh", [128, NWIN * kj * 128], u16, kind="ExternalInput")
    ohT_d = nc.dram_tensor("ohT", [128, NWIN * kj * 128], u16, kind="ExternalInput")
    out_d = nc.dram_tensor("out", [WROWS, HD], f32, kind="ExternalOutput")

    X = mybir.AxisListType.X
    MAX = mybir.AluOpType.max

    with tile.TileContext(nc) as tc:
        with tc.tile_pool(name="const", bufs=1) as cpool:
            spsum_cm = tc.tile_pool(name="setup_psum", bufs=1, space="PSUM")
            spsum = spsum_cm.__enter__()
            ones = cpool.tile([1, 128], f32)
            nc.vector.memset(ones[:], 1.0)

            # WA = [W | Wsrc | 0.2*Wsrc], Wsrc[k,h] = sum_d W[k,h*D+d]*att_src[h,d];
            # Wdst likewise. The 0.2-scaled twin columns make the score psum
            # hold [s | 0.2*s], so leaky_relu is ONE reduce_max over the pair.
            wa_tmp = cpool.tile([IN_DIM, HD + SC2], f32)
            nc.vector.memset(wa_tmp[:], 0.0)
            nc.sync.dma_start(wa_tmp[:, 0:HD], W_d[:, :])
            wdst = cpool.tile([IN_DIM, SC2], f32)
            att_s_raw = cpool.tile([1, HD], f32)
            nc.sync.dma_start(att_s_raw[:], asrc_d[:, :])
            att_t_raw = cpool.tile([1, HD], f32)
            nc.sync.dma_start(att_t_raw[:], adst_d[:, :])
            att_s = cpool.tile([1, HD], f32)
            nc.vector.tensor_copy(att_s[:], att_s_raw[:])
            att_t = cpool.tile([1, HD], f32)
            nc.vector.tensor_copy(att_t[:], att_t_raw[:])
            for att_tile, dst_ap in ((att_s, wa_tmp[:, HD : HD + H]), (att_t, wdst[:, 0:H])):
                attb = spsum.tile([IN_DIM, HD], f32, tag="attb")
                nc.tensor.matmul(
                    attb[:], lhsT=ones[:1, 0:IN_DIM], rhs=att_tile[:],
                    start=True, stop=True,
                )
                tmp = cpool.tile([IN_DIM, HD], f32, tag="tmp")
                nc.vector.tensor_mul(tmp[:], wa_tmp[:, 0:HD], attb[:])
                nc.vector.reduce_sum(
                    out=dst_ap,
                    in_=tmp[:].rearrange("k (h d) -> k h d", d=D),
                    axis=X,
                )
            nc.vector.tensor_scalar_mul(
                wa_tmp[:, HD + H : HD + SC2], wa_tmp[:, HD : HD + H], NEG_SLOPE
            )
            nc.vector.tensor_scalar_mul(wdst[:, H:SC2], wdst[:, 0:H], NEG_SLOPE)
            WAb = cpool.tile([IN_DIM, HD + SC2], bf16)
            nc.vector.tensor_copy(WAb[:], wa_tmp[:])

            bias_raw = cpool.tile([1, HD], f32)
            nc.sync.dma_start(bias_raw[:], bias_d[:, :])
            bias_sb = cpool.tile([1, HD], f32)
            nc.vector.tensor_copy(bias_sb[:], bias_raw[:])
            bb = spsum.tile([128, HD], f32)
            nc.tensor.matmul(bb[:], lhsT=ones[:1, :], rhs=bias_sb[:], start=True, stop=True)
            bias_bc = cpool.tile([128, HD], f32)
            nc.scalar.copy(bias_bc[:], bb[:])

            # phase 1b: [a_dst | 0.2*a_dst] per dst shard, bf16 [128, NWIN*SC2]
            adst_all = cpool.tile([128, NWIN * SC2], bf16)

            # preloaded per-window index data (batched DMAs -- per-DMA fixed
            # cost on HW is ~2us, so per-window loads are expensive)
            il_all = cpool.tile([128, NWIN * KL * 8], i16)
            nc.sync.dma_start(il_all[:], il_d[:, :])
            ih_all = cpool.tile([128, NWIN * KH * 8], i16)
            nc.sync.dma_start(ih_all[:], ih_d[:, :])

            spsum_cm.__exit__(None, None, None)  # free setup PSUM banks

            P1B = 8  # windows per psum batch
            with (
                tc.tile_pool(name="p1", bufs=1) as p1,
                tc.tile_pool(name="p1ps", bufs=2, space="PSUM") as p1ps,
            ):
                xd = p1.tile([IN_DIM, WROWS], f32, tag="xdr")
                nc.sync.dma_start(xd[:], xdT_d[:, :])
                for w0 in range(0, NWIN, P1B):
                    nb = min(P1B, NWIN - w0)
                    adp = p1ps.tile([128, P1B * SC2], f32, tag="adp")
                    for k in range(nb):
                        w = w0 + k
                        nc.tensor.matmul(
                            adp[:, k * SC2 : (k + 1) * SC2],
                            lhsT=xd[:, w * 128 : (w + 1) * 128],
                            rhs=wdst[:], start=True, stop=True,
                        )
                    nc.vector.tensor_copy(
                        adst_all[:, w0 * SC2 : (w0 + nb) * SC2], adp[:, 0 : nb * SC2]
                    )

            # ---------------- main loop: per-window aggregation ----------------
            if ablate == "p1":
                with tc.tile_pool(name="fin0", bufs=1) as f0:
                    zo = f0.tile([128, HD], f32)
                    nc.vector.memset(zo[:], 0.0)
                    for w in range(NWIN):
                        nc.sync.dma_start(out_d[w * 128 : (w + 1) * 128, :], zo[:])
            nch = math.ceil(kj / SUBS)
            OB = 7  # windows per output-write batch (49 = 7*7)
            LOOKAHEAD = 1  # chunks of PE h/asr/adx emitted ahead of acc
            with (
                tc.tile_pool(name="gat", bufs=3) as gpool,
                tc.tile_pool(name="ohp", bufs=2) as ohpool,
                tc.tile_pool(name="wrk", bufs=6) as wpool,
                tc.tile_pool(name="fin", bufs=2) as fpool,
                tc.tile_pool(name="hps", bufs=2, space="PSUM") as hpool,
                tc.tile_pool(name="sps", bufs=2, space="PSUM") as spool,
                tc.tile_pool(name="acc", bufs=2, space="PSUM") as apool,
            ):
                for g0 in range(0, NWIN if ablate != "p1" else 0, OB):
                  gnb = min(OB, NWIN - g0)
                  outb = fpool.tile([128, OB * HD], f32, tag="outb")
                  for k in range(gnb):
                    w = g0 + k
                    g = gpool.tile([128, kj * 128], u16, tag="g")
                    gv = g[:].unsqueeze(1)  # [128, 1, kj*128]
                    nc.gpsimd.dma_gather(
                        out_ap=gv[:, :, 0 : KL * 128], in_ap=xlo_d[:, :],
                        idxs_ap=il_all[:, w * KL * 8 : (w + 1) * KL * 8],
                        num_idxs=KL * 128, num_idxs_reg=KL * 128,
                        elem_size=128, transpose=True, single_packet=False,
                    )
                    nc.gpsimd.dma_gather(
                        out_ap=gv[:, :, KL * 128 : kj * 128], in_ap=xhi_d[:, :],
                        idxs_ap=ih_all[:, w * KH * 8 : (w + 1) * KH * 8],
                        num_idxs=KH * 128, num_idxs_reg=KH * 128,
                        elem_size=128, transpose=True, single_packet=False,
                    )
                    gb = g[:].bitcast(bf16)
                    adw = adst_all[:, w * SC2 : (w + 1) * SC2]

                    if ablate == "p1g":
                        nc.vector.memset(outb[:, k * HD : (k + 1) * HD], 0.0)
                        nc.vector.tensor_copy(
                            outb[0:IN_DIM, k * HD : k * HD + HD],
                            gb[0:IN_DIM, 0:HD],
                        )
                        if k == gnb - 1:
                            nc.sync.dma_start(
                                out_d[g0 * 128 : (g0 + gnb) * 128, :].rearrange(
                                    "(k p) c -> p k c", p=128
                                ),
                                outb[:, 0 : gnb * HD].rearrange(
                                    "p (k c) -> p k c", c=HD
                                ),
                            )
                        continue

                    accdns = apool.tile([128, WCOLS], f32, tag="accdns")

                    # host-built one-hots, both layouts, streamed per window
                    oh_u = ohpool.tile([128, kj * 128], u16, tag="oh")
                    nc.sync.dma_start(
                        oh_u[:], oh_d[:, w * kj * 128 : (w + 1) * kj * 128]
                    )
                    ohT_u = ohpool.tile([128, kj * 128], u16, tag="ohT")
                    nc.sync.dma_start(
                        ohT_u[:], ohT_d[:, w * kj * 128 : (w + 1) * kj * 128]
                    )
                    oh_all = oh_u[:].bitcast(bf16)
                    ohT_all = ohT_u[:].bitcast(bf16)

                    # --- chunk phase, software-pipelined PE stream ---
                    # emit_h(c): projection h (1-bank psum) + score psum
                    # (a_src matmul accumulated with a_dst matmul)
                    def emit_h(c):
                        s0 = c * SUBS
                        ns = min(SUBS, kj - s0)
                        hps = hpool.tile([128, SUBS * HD], f32, tag="hps")
                        scps = spool.tile([128, SUBS * SC2], f32, tag="scps")
                        for s in range(ns):
                            sub = gb[
                                0:IN_DIM, (s0 + s) * 128 : (s0 + s + 1) * 128
                            ]
                            nc.tensor.matmul(
                                hps[:, s * HD : (s + 1) * HD],
                                lhsT=sub, rhs=WAb[:, 0:HD],
                                start=True, stop=True,
                            )
                            nc.tensor.matmul(
                                scps[:, s * SC2 : (s + 1) * SC2],
                                lhsT=sub, rhs=WAb[:, HD : HD + SC2],
                                start=True, stop=False,
                            )
                            nc.tensor.matmul(
                                scps[:, s * SC2 : (s + 1) * SC2],
                                lhsT=ohT_all[
                                    :, (s0 + s) * 128 : (s0 + s + 1) * 128
                                ],
                                rhs=adw,
                                start=False, stop=True,
                            )
                        return hps, scps, s0, ns

                    pending = [emit_h(c) for c in range(min(LOOKAHEAD, nch))]
                    for ch in range(nch):
                        if ch + LOOKAHEAD < nch:
                            pending.append(emit_h(ch + LOOKAHEAD))
                        hps, scps, s0, ns = pending[ch]
                        # msg[:, s, 0:HD] = h_s * e ; msg[:, s, HD:HD+H] = e
                        # exp writes the e columns of msg DIRECTLY (no copy op)
                        msg = wpool.tile([128, SUBS * WCOLS], bf16, tag="msg")
                        mv = msg[:].rearrange("p (s c) -> p s c", s=SUBS)
                        # leaky_relu = max(s, 0.2*s): the score psum already
                        # holds [s | 0.2*s] per subtile, so ONE reduce_max
                        # over the pair axis (single PSUM input AP) does it
                        lrx = wpool.tile([128, SUBS * H], f32, tag="lrx")
                        nc.vector.reduce_max(
                            out=lrx[:, 0 : ns * H].rearrange(
                                "p (s h) -> p s h", h=H
                            ),
                            in_=scps[:, 0 : ns * SC2].rearrange(
                                "p (s a h) -> p s h a", a=2, h=H
                            ),
                            axis=X,
                        )
                        exf = wpool.tile([128, SUBS * H], f32, tag="exf")
                        nc.scalar.activation(
                            exf[:, 0 : ns * H],
                            lrx[:, 0 : ns * H],
                            mybir.ActivationFunctionType.Exp,
                        )
                        # e columns of msg (for the denominator matmul) --
                        # off the critical path
                        nc.scalar.copy(
                            mv[:, 0:ns, HD : HD + H],
                            exf[:, 0 : ns * H].rearrange("p (s h) -> p s h", h=H),
                        )
                        exv = exf[:, 0 : ns * H].rearrange("p (s h) -> p s h", h=H)
                        # h*e split: DVE scales heads 0:3, Act scales head 3
                        # (per-subtile scalar-AP mul straight from PSUM) --
                        # DVE is the bottleneck engine, Act is nearly idle
                        HS = 3 * D  # 192: cols of the DVE-scaled heads
                        nc.vector.tensor_mul(
                            mv[:, 0:ns, 0:HS].rearrange("p s (h d) -> p s h d", d=D),
                            hps[:]
                            .rearrange("p (s c) -> p s c", c=HD)[:, 0:ns, 0:HS]
                            .rearrange("p s (h d) -> p s h d", d=D),
                            exv[:, :, 0:3]
                            .unsqueeze(-1)
                            .to_broadcast([128, ns, 3, D]),
                        )
                        for s in range(ns):
                            nc.scalar.mul(
                                mv[:, s, HS:HD],
                                hps[:, s * HD + HS : s * HD + HD],
                                exf[:, s * H + 3 : s * H + 4],
                            )
                        # accumulate [sum(e*h) | sum(e)] over the window
                        for s in range(ns):
                            q = s0 + s
                            nc.tensor.matmul(
                                accdns[:],
                                lhsT=oh_all[:, (s0 + s) * 128 : (s0 + s + 1) * 128],
                                rhs=mv[:, s, :],
                                start=(q == 0),
                                stop=(q == kj - 1),
                            )
                    # finalize: out = acc / (dns + eps) + bias
                    dnse = fpool.tile([128, H], f32, tag="dnse")
                    nc.vector.tensor_scalar_add(dnse[:], accdns[:, HD : HD + H], EPS)
                    dnr = fpool.tile([128, H], f32, tag="dnr")
                    nc.vector.reciprocal(dnr[:], dnse[:])
                    outw = outb[:, k * HD : (k + 1) * HD]
                    nc.vector.tensor_mul(
                        outw.rearrange("p (h d) -> p h d", d=D),
                        accdns[:, 0:HD].rearrange("p (h d) -> p h d", d=D),
                        dnr[:].unsqueeze(-1).to_broadcast([128, H, D]),
                    )
                    nc.vector.tensor_add(outw, outw, bias_bc[:])
                    if k == gnb - 1:
                        nc.sync.dma_start(
                            out_d[g0 * 128 : (g0 + gnb) * 128, :].rearrange(
                                "(k p) c -> p k c", p=128
                            ),
                            outb[:, 0 : gnb * HD].rearrange("p (k c) -> p k c", c=HD),
                        )
    nc.compile()
    # compile()'s late passes (act-table loads, hostgen rebases) can leave
    # >1-wait instructions behind; one more split pass clears them (the TRN2
    # ISA allows a single sem wait per compute instruction).
    nc.generate_event_semaphores()
    return nc


def _stage_inputs(x, W, att_src, att_dst, bias, ilow, ihigh, oh_u, ohT_u):
    x = np.asarray(x, dtype=np.float32)
    x_il = _interleave_x(x)
    x_lo = np.ascontiguousarray(x_il[:SPLIT])
    x_hi = np.ascontiguousarray(x_il[SPLIT:])
    asrc_row = np.ascontiguousarray(np.asarray(att_src, np.float32).reshape(1, HD))
    adst_row = np.ascontiguousarray(np.asarray(att_dst, np.float32).reshape(1, HD))
    bias_row = np.ascontiguousarray(np.asarray(bias, np.float32).reshape(1, HD))
    in_maps = []
    for c in range(NCORES):
        xdT = np.zeros((IN_DIM, WROWS), dtype=np.float32)
        xdT[:, :NPC] = x[c * NPC : (c + 1) * NPC].T
        in_maps.append(
            {
                "x_lo": x_lo,
                "x_hi": x_hi,
                "xdstT": np.ascontiguousarray(xdT),
                "W": np.asarray(W, np.float32),
                "att_src": asrc_row,
                "att_dst": adst_row,
                "bias": bias_row,
                "ilow": np.ascontiguousarray(
                    ilow[c].transpose(1, 0, 2).reshape(128, -1)
                ),
                "ihigh": np.ascontiguousarray(
                    ihigh[c].transpose(1, 0, 2).reshape(128, -1)
                ),
                "oh": np.ascontiguousarray(oh_u[c]),
                "ohT": np.ascontiguousarray(ohT_u[c]),
            }
        )
    return in_maps


def kernel(x, edge_index, W, att_src, att_dst, bias):
    global LAST_RESULTS
    from concourse.bass_utils import run_bass_kernel_spmd

    edge_index = np.asarray(edge_index)
    ilow, ihigh, oh_u, ohT_u, KL, KH = _prep_host(edge_index)
    nc = _build_program(KL, KH)
    in_maps = _stage_inputs(x, W, att_src, att_dst, bias, ilow, ihigh, oh_u, ohT_u)

    res = run_bass_kernel_spmd(nc, in_maps, list(range(NCORES)))
    LAST_RESULTS = res

    out = np.empty((N, HD), dtype=np.float32)
    for c in range(NCORES):
        out[c * NPC : (c + 1) * NPC] = res.results[c]["out"][:NPC]
    return out


# revision 51
# speedup vs baseline: 1.7995x; 1.7995x over previous
"""GAT layer (PyG-style, add_self_loops=True) on 8 Trainium2 NeuronCores.

Strategy: partition destination nodes (and their incident edges) across the 8
cores; each core owns a contiguous range of 6250 dst nodes (49 windows of 128).

No projection table. Per window of 128 dst nodes, two transposed dma_gathers
(lo/hi halves of the node range, int16-index limit) pull the raw 256-byte x
rows of all incident edges' sources straight out of HBM, TRANSPOSED at u16
granularity: the host pre-interleaves each x row's bytes as
[hi16(x_0)..hi16(x_63) | lo16(x_0)..lo16(x_63)], so gather partitions 0:64
hold truncated-bf16 features and serve directly as the matmul lhsT. Each
128-edge subtile then computes h|a_src = x_src @ [W | W@att_src^T] as ONE bf16
matmul (f32 PSUM), so there is no replicated 50k-row projection pass and no
38 MB table write at all.

The edge->slot one-hot matrices are pure index data, so the HOST builds them
(both layouts: edge-major for the segment-sum lhsT, slot-major for the a_dst
expansion) as bf16 bit patterns, streamed in per window by DMA -- no DVE
one-hot build, no PE transposes, no PSUM->SBUF copies. a_src accumulates with
a_dst (ohT_w @ a_dst_window matmul) into one tiny score PSUM via start/stop
chaining, with 0.2-scaled twin weight columns so the score psum holds
[s | 0.2*s] and leaky_relu is a single DVE reduce_max over the pair; Act exp
writes the e-columns of the msg tile directly; DVE fuses the PSUM->SBUF h move
with the e-scale
(msg = h*e, bf16). Segment-sum of [e*h | e] via PSUM-accumulated one-hot
matmuls, software-pipelined so projection matmuls run a chunk ahead of the
accumulation matmuls; final out = acc/(denom+eps) + bias, batched 7 windows
per output DMA. Softmax max-subtraction is skipped (shift-invariant; scores
are O(1) so exp cannot overflow).

Pad edges gather row 0 (finite) and carry dst-slot sentinel 500 => their
one-hot row is all zero, so they contribute to nothing. No dummy rows.

All small per-window inputs (gather indices, one-hots) are laid out
partition-major on the host so every DMA is a few large contiguous
descriptors -- per-DMA fixed cost (~2us DGE+sem) dominated the v1 kernel.

Host does only index/byte-space work (self-loop append, dst sort, windowing,
padding, int16 index wrapping, u16 byte interleave of x, one-hot bit
patterns, x transpose/slice).

Act-engine Lrelu was tried for the leaky_relu and produces WRONG results on
HW (semantics differ from alpha*min(x,0)+max(x,0)); keep the DVE max-pair.
"""

import math

import numpy as np

N = 50000
IN_DIM = 64
H = 4
D = 64
HD = H * D  # 256
WCOLS = HD + H  # 260: msg cols h | e
SC2 = 2 * H  # score psum cols per subtile: [s | 0.2*s] (leaky via reduce_max)
NEG_SLOPE = 0.2
EPS = 1e-16
SENT = 500.0  # dst-slot sentinel for pad edges (one-hot row all zero)

NCORES = 8
NPC = N // NCORES  # 6250 dst nodes per core
NWIN = math.ceil(NPC / 128)  # 49 windows
WROWS = NWIN * 128  # 6272
SPLIT = 25088  # lo/hi x-table split (int16 gather index limit)
SUBS = 4  # edge subtiles per chunk (PSUM: hps 2 banks x2, spt 1 bank x2, acc 2)
USE_ACT_LRELU = False  # Act-engine Lrelu gives WRONG results on HW; keep DVE

LAST_RESULTS = None  # BassKernelResults of the most recent run (for test.py)


def _wrap_idx(ids):
    """[n] int -> dma_gather wrapped layout [128, n/16] int16
    (idx i at [i%16, i//16], replicated across the 8 Q7 core groups)."""
    n = len(ids)
    w16 = ids.reshape(n // 16, 16).T.astype(np.int16)  # [16, n/16]
    return np.tile(w16, (8, 1))


def _interleave_x(x):
    """[N,64] f32 -> [N,128] u16 rows [hi16(x_0..63) | lo16(x_0..63)].
    After the u16-granularity transposed gather, partitions 0:64 hold the
    high halves = truncated-bf16 feature values."""
    xu = np.ascontiguousarray(x).view(np.uint16).reshape(-1, 64, 2)
    return np.ascontiguousarray(np.concatenate([xu[:, :, 1], xu[:, :, 0]], axis=1))


def _prep_host(edge_index):
    """Returns ilow  int16 [NCORES, NWIN, 128, KL*8]
               ihigh int16 [NCORES, NWIN, 128, KH*8]
               dstrel f32  [NCORES, NWIN, 128, KL+KH]  (slot or SENT)
               (KL, KH)"""
    src = np.concatenate([edge_index[0], np.arange(N, dtype=np.int64)]).astype(np.int64)
    dst = np.concatenate([edge_index[1], np.arange(N, dtype=np.int64)]).astype(np.int64)
    order = np.argsort(dst, kind="stable")
    src = src[order].astype(np.int32)
    dst = dst[order].astype(np.int32)

    bounds = [c * NPC + w * 128 for c in range(NCORES) for w in range(NWIN)]
    bounds.append(N)
    cuts = np.searchsorted(dst, np.asarray(bounds))

    lo_counts = np.zeros(NCORES * NWIN, np.int64)
    hi_counts = np.zeros(NCORES * NWIN, np.int64)
    for b in range(NCORES * NWIN):
        s = src[cuts[b] : cuts[b + 1]]
        lo_counts[b] = int((s < SPLIT).sum())
        hi_counts[b] = len(s) - lo_counts[b]
    KL = max(1, math.ceil(lo_counts.max() / 128))
    KH = max(1, math.ceil(hi_counts.max() / 128))
    kj = KL + KH

    ilow = np.zeros((NCORES, NWIN, KL * 128), np.int32)  # pad -> row 0 (finite)
    ihigh = np.zeros((NCORES, NWIN, KH * 128), np.int32)
    dstrel = np.full((NCORES, NWIN, 128, kj), SENT, np.float32)
    for c in range(NCORES):
        base = c * NPC
        for w in range(NWIN):
            b = c * NWIN + w
            s = src[cuts[b] : cuts[b + 1]]
            d = dst[cuts[b] : cuts[b + 1]] - base - w * 128
            m = s < SPLIT
            slo, dlo = s[m], d[m]
            shi, dhi = s[~m] - SPLIT, d[~m]
            # ascending source rows => HBM page locality in the gather
            o = np.argsort(slo, kind="stable")
            slo, dlo = slo[o], dlo[o]
            o = np.argsort(shi, kind="stable")
            shi, dhi = shi[o], dhi[o]
            ilow[c, w, : len(slo)] = slo
            ihigh[c, w, : len(shi)] = shi
            i = np.arange(len(slo))
            dstrel[c, w, i % 128, i // 128] = dlo
            i = np.arange(len(shi))
            dstrel[c, w, i % 128, KL + i // 128] = dhi
    ilow_w = np.zeros((NCORES, NWIN, 128, KL * 8), np.int16)
    ihigh_w = np.zeros((NCORES, NWIN, 128, KH * 8), np.int16)
    for c in range(NCORES):
        for w in range(NWIN):
            ilow_w[c, w] = _wrap_idx(ilow[c, w])
            ihigh_w[c, w] = _wrap_idx(ihigh[c, w])

    # one-hot (edge->slot) matrices in BOTH layouts as bf16 bit patterns
    # (0x3F80 = bf16 1.0), partition-major for contiguous per-window DMA:
    #   oh [p=e, w*kj*128 + s*128 + slot], ohT [p=slot, w*kj*128 + s*128 + e]
    one = np.uint16(0x3F80)
    slots = np.arange(128)
    oh_u = np.zeros((NCORES, 128, NWIN * kj * 128), np.uint16)
    ohT_u = np.zeros((NCORES, 128, NWIN * kj * 128), np.uint16)
    for c in range(NCORES):
        eq = dstrel[c][:, :, :, None] == slots  # [w, e, s, slot] bool
        oh_u[c] = (
            eq.transpose(1, 0, 2, 3).reshape(128, -1) * one
        )
        ohT_u[c] = (
            eq.transpose(3, 0, 2, 1).reshape(128, -1) * one
        )
    return ilow_w, ihigh_w, oh_u, ohT_u, KL, KH


def _build_program(KL, KH, ablate="full"):
    import concourse.bass as bass
    import concourse.bacc as bacc
    import concourse.tile as tile
    from concourse import mybir

    f32 = mybir.dt.float32
    bf16 = mybir.dt.bfloat16
    u16 = mybir.dt.uint16
    i16 = mybir.dt.int16
    kj = KL + KH

    nc = bacc.Bacc(None, target_bir_lowering=False)

    xlo_d = nc.dram_tensor("x_lo", [SPLIT, 128], u16, kind="ExternalInput")
    xhi_d = nc.dram_tensor("x_hi", [N - SPLIT, 128], u16, kind="ExternalInput")
    xdT_d = nc.dram_tensor("xdstT", [IN_DIM, WROWS], f32, kind="ExternalInput")
    W_d = nc.dram_tensor("W", [IN_DIM, HD], f32, kind="ExternalInput")
    asrc_d = nc.dram_tensor("att_src", [1, HD], f32, kind="ExternalInput")
    adst_d = nc.dram_tensor("att_dst", [1, HD], f32, kind="ExternalInput")
    bias_d = nc.dram_tensor("bias", [1, HD], f32, kind="ExternalInput")
    # partition-major on host so the one-shot preload DMA is contiguous per
    # partition (128 large descriptors, not NWIN*128 small ones)
    il_d = nc.dram_tensor("ilow", [128, NWIN * KL * 8], i16, kind="ExternalInput")
    ih_d = nc.dram_tensor("ihigh", [128, NWIN * KH * 8], i16, kind="ExternalInput")
    # host-built one-hot matrices (bf16 bit patterns), both layouts
    oh_d = nc.dram_tensor("oh", [128, NWIN * kj * 128], u16, kind="ExternalInput")
    ohT_d = nc.dram_tensor("ohT", [128, NWIN * kj * 128], u16, kind="ExternalInput")
    out_d = nc.dram_tensor("out", [WROWS, HD], f32, kind="ExternalOutput")

    X = mybir.AxisListType.X
    MAX = mybir.AluOpType.max

    with tile.TileContext(nc) as tc:
        with tc.tile_pool(name="const", bufs=1) as cpool:
            spsum_cm = tc.tile_pool(name="setup_psum", bufs=1, space="PSUM")
            spsum = spsum_cm.__enter__()
            ones = cpool.tile([1, 128], f32)
            nc.vector.memset(ones[:], 1.0)

            # WA = [W | Wsrc | 0.2*Wsrc], Wsrc[k,h] = sum_d W[k,h*D+d]*att_src[h,d];
            # Wdst likewise. The 0.2-scaled twin columns make the score psum
            # hold [s | 0.2*s], so leaky_relu is ONE reduce_max over the pair.
            wa_tmp = cpool.tile([IN_DIM, HD + SC2], f32)
            nc.vector.memset(wa_tmp[:], 0.0)
            nc.sync.dma_start(wa_tmp[:, 0:HD], W_d[:, :])
            wdst = cpool.tile([IN_DIM, SC2], f32)
            att_s_raw = cpool.tile([1, HD], f32)
            nc.sync.dma_start(att_s_raw[:], asrc_d[:, :])
            att_t_raw = cpool.tile([1, HD], f32)
            nc.sync.dma_start(att_t_raw[:], adst_d[:, :])
            att_s = cpool.tile([1, HD], f32)
            nc.vector.tensor_copy(att_s[:], att_s_raw[:])
            att_t = cpool.tile([1, HD], f32)
            nc.vector.tensor_copy(att_t[:], att_t_raw[:])
            for att_tile, dst_ap in ((att_s, wa_tmp[:, HD : HD + H]), (att_t, wdst[:, 0:H])):
                attb = spsum.tile([IN_DIM, HD], f32, tag="attb")
                nc.tensor.matmul(
                    attb[:], lhsT=ones[:1, 0:IN_DIM], rhs=att_tile[:],
                    start=True, stop=True,
                )
                tmp = cpool.tile([IN_DIM, HD], f32, tag="tmp")
                nc.vector.tensor_mul(tmp[:], wa_tmp[:, 0:HD], attb[:])
                nc.vector.reduce_sum(
                    out=dst_ap,
                    in_=tmp[:].rearrange("k (h d) -> k h d", d=D),
                    axis=X,
                )
            nc.vector.tensor_scalar_mul(
                wa_tmp[:, HD + H : HD + SC2], wa_tmp[:, HD : HD + H], NEG_SLOPE
            )
            nc.vector.tensor_scalar_mul(wdst[:, H:SC2], wdst[:, 0:H], NEG_SLOPE)
            WAb = cpool.tile([IN_DIM, HD + SC2], bf16)
            nc.vector.tensor_copy(WAb[:], wa_tmp[:])

            bias_raw = cpool.tile([1, HD], f32)
            nc.sync.dma_start(bias_raw[:], bias_d[:, :])
            bias_sb = cpool.tile([1, HD], f32)
            nc.vector.tensor_copy(bias_sb[:], bias_raw[:])
            bb = spsum.tile([128, HD], f32)
            nc.tensor.matmul(bb[:], lhsT=ones[:1, :], rhs=bias_sb[:], start=True, stop=True)
            bias_bc = cpool.tile([128, HD], f32)
            nc.scalar.copy(bias_bc[:], bb[:])

            # phase 1b: [a_dst | 0.2*a_dst] per dst shard, bf16 [128, NWIN*SC2]
            adst_all = cpool.tile([128, NWIN * SC2], bf16)

            # preloaded per-window index data (batched DMAs -- per-DMA fixed
            # cost on HW is ~2us, so per-window loads are expensive)
            il_all = cpool.tile([128, NWIN * KL * 8], i16)
            nc.sync.dma_start(il_all[:], il_d[:, :])
            ih_all = cpool.tile([128, NWIN * KH * 8], i16)
            nc.sync.dma_start(ih_all[:], ih_d[:, :])

            spsum_cm.__exit__(None, None, None)  # free setup PSUM banks

            P1B = 8  # windows per psum batch
            with (
                tc.tile_pool(name="p1", bufs=1) as p1,
                tc.tile_pool(name="p1ps", bufs=2, space="PSUM") as p1ps,
            ):
                xd = p1.tile([IN_DIM, WROWS], f32, tag="xdr")
                nc.sync.dma_start(xd[:], xdT_d[:, :])
                for w0 in range(0, NWIN, P1B):
                    nb = min(P1B, NWIN - w0)
                    adp = p1ps.tile([128, P1B * SC2], f32, tag="adp")
                    for k in range(nb):
                        w = w0 + k
                        nc.tensor.matmul(
                            adp[:, k * SC2 : (k + 1) * SC2],
                            lhsT=xd[:, w * 128 : (w + 1) * 128],
                            rhs=wdst[:], start=True, stop=True,
                        )
                    nc.vector.tensor_copy(
                        adst_all[:, w0 * SC2 : (w0 + nb) * SC2], adp[:, 0 : nb * SC2]
                    )

            # ---------------- main loop: per-window aggregation ----------------
            if ablate == "p1":
                with tc.tile_pool(name="fin0", bufs=1) as f0:
                    zo = f0.tile([128, HD], f32)
                    nc.vector.memset(zo[:], 0.0)
                    for w in range(NWIN):
                        nc.sync.dma_start(out_d[w * 128 : (w + 1) * 128, :], zo[:])
            nch = math.ceil(kj / SUBS)
            OB = 7  # windows per output-write batch (49 = 7*7)
            LOOKAHEAD = 1  # chunks of PE h/asr/adx emitted ahead of acc
            with (
                tc.tile_pool(name="gat", bufs=4) as gpool,
                tc.tile_pool(name="ohp", bufs=3) as ohpool,
                tc.tile_pool(name="wrk", bufs=6) as wpool,
                tc.tile_pool(name="fin", bufs=2) as fpool,
                tc.tile_pool(name="hps", bufs=2, space="PSUM") as hpool,
                tc.tile_pool(name="sps", bufs=2, space="PSUM") as spool,
                tc.tile_pool(name="acc", bufs=2, space="PSUM") as apool,
            ):
                for g0 in range(0, NWIN if ablate != "p1" else 0, OB):
                  gnb = min(OB, NWIN - g0)
                  outb = fpool.tile([128, OB * HD], f32, tag="outb")
                  for k in range(gnb):
                    w = g0 + k
                    g = gpool.tile([128, kj * 128], u16, tag="g")
                    gv = g[:].unsqueeze(1)  # [128, 1, kj*128]
                    nc.gpsimd.dma_gather(
                        out_ap=gv[:, :, 0 : KL * 128], in_ap=xlo_d[:, :],
                        idxs_ap=il_all[:, w * KL * 8 : (w + 1) * KL * 8],
                        num_idxs=KL * 128, num_idxs_reg=KL * 128,
                        elem_size=128, transpose=True, single_packet=False,
                    )
                    nc.gpsimd.dma_gather(
                        out_ap=gv[:, :, KL * 128 : kj * 128], in_ap=xhi_d[:, :],
                        idxs_ap=ih_all[:, w * KH * 8 : (w + 1) * KH * 8],
                        num_idxs=KH * 128, num_idxs_reg=KH * 128,
                        elem_size=128, transpose=True, single_packet=False,
                    )
                    gb = g[:].bitcast(bf16)
                    adw = adst_all[:, w * SC2 : (w + 1) * SC2]

                    if ablate == "p1g":
                        nc.vector.memset(outb[:, k * HD : (k + 1) * HD], 0.0)
                        nc.vector.tensor_copy(
                            outb[0:IN_DIM, k * HD : k * HD + HD],
                            gb[0:IN_DIM, 0:HD],
                        )
                        if k == gnb - 1:
                            nc.sync.dma_start(
                                out_d[g0 * 128 : (g0 + gnb) * 128, :].rearrange(
                                    "(k p) c -> p k c", p=128
                                ),
                                outb[:, 0 : gnb * HD].rearrange(
                                    "p (k c) -> p k c", c=HD
                                ),
                            )
                        continue

                    accdns = apool.tile([128, WCOLS], f32, tag="accdns")

                    # host-built one-hots, both layouts, streamed per window
                    oh_u = ohpool.tile([128, kj * 128], u16, tag="oh")
                    nc.sync.dma_start(
                        oh_u[:], oh_d[:, w * kj * 128 : (w + 1) * kj * 128]
                    )
                    ohT_u = ohpool.tile([128, kj * 128], u16, tag="ohT")
                    nc.sync.dma_start(
                        ohT_u[:], ohT_d[:, w * kj * 128 : (w + 1) * kj * 128]
                    )
                    oh_all = oh_u[:].bitcast(bf16)
                    ohT_all = ohT_u[:].bitcast(bf16)

                    # --- chunk phase, software-pipelined PE stream ---
                    # emit_h(c): projection h (1-bank psum) + score psum
                    # (a_src matmul accumulated with a_dst matmul)
                    def emit_h(c):
                        s0 = c * SUBS
                        ns = min(SUBS, kj - s0)
                        hps = hpool.tile([128, SUBS * HD], f32, tag="hps")
                        scps = spool.tile([128, SUBS * SC2], f32, tag="scps")
                        for s in range(ns):
                            sub = gb[
                                0:IN_DIM, (s0 + s) * 128 : (s0 + s + 1) * 128
                            ]
                            nc.tensor.matmul(
                                hps[:, s * HD : (s + 1) * HD],
                                lhsT=sub, rhs=WAb[:, 0:HD],
                                start=True, stop=True,
                            )
                            nc.tensor.matmul(
                                scps[:, s * SC2 : (s + 1) * SC2],
                                lhsT=sub, rhs=WAb[:, HD : HD + SC2],
                                start=True, stop=False,
                            )
                            nc.tensor.matmul(
                                scps[:, s * SC2 : (s + 1) * SC2],
                                lhsT=ohT_all[
                                    :, (s0 + s) * 128 : (s0 + s + 1) * 128
                                ],
                                rhs=adw,
                                start=False, stop=True,
                            )
                        return hps, scps, s0, ns

                    pending = [emit_h(c) for c in range(min(LOOKAHEAD, nch))]
                    for ch in range(nch):
                        if ch + LOOKAHEAD < nch:
                            pending.append(emit_h(ch + LOOKAHEAD))
                        hps, scps, s0, ns = pending[ch]
                        # msg[:, s, 0:HD] = h_s * e ; msg[:, s, HD:HD+H] = e
                        # exp writes the e columns of msg DIRECTLY (no copy op)
                        msg = wpool.tile([128, SUBS * WCOLS], bf16, tag="msg")
                        mv = msg[:].rearrange("p (s c) -> p s c", s=SUBS)
                        # leaky_relu = max(s, 0.2*s): the score psum already
                        # holds [s | 0.2*s] per subtile, so ONE reduce_max
                        # over the pair axis (single PSUM input AP) does it
                        lrx = wpool.tile([128, SUBS * H], f32, tag="lrx")
                        nc.vector.reduce_max(
                            out=lrx[:, 0 : ns * H].rearrange(
                                "p (s h) -> p s h", h=H
                            ),
                            in_=scps[:, 0 : ns * SC2].rearrange(
                                "p (s a h) -> p s h a", a=2, h=H
                            ),
                            axis=X,
                        )
                        exf = wpool.tile([128, SUBS * H], f32, tag="exf")
                        nc.scalar.activation(
                            exf[:, 0 : ns * H],
                            lrx[:, 0 : ns * H],
                            mybir.ActivationFunctionType.Exp,
                        )
                        # e columns of msg (for the denominator matmul) --
                        # off the critical path
                        nc.scalar.copy(
                            mv[:, 0:ns, HD : HD + H],
                            exf[:, 0 : ns * H].rearrange("p (s h) -> p s h", h=H),
                        )
                        exv = exf[:, 0 : ns * H].rearrange("p (s h) -> p s h", h=H)
                        # h*e split: DVE scales heads 0:3, Act scales head 3
                        # (per-subtile scalar-AP mul straight from PSUM) --
                        # DVE is the bottleneck engine, Act is nearly idle
                        HS = 3 * D  # 192: cols of the DVE-scaled heads
                        nc.vector.tensor_mul(
                            mv[:, 0:ns, 0:HS].rearrange("p s (h d) -> p s h d", d=D),
                            hps[:]
                            .rearrange("p (s c) -> p s c", c=HD)[:, 0:ns, 0:HS]
                            .rearrange("p s (h d) -> p s h d", d=D),
                            exv[:, :, 0:3]
                            .unsqueeze(-1)
                            .to_broadcast([128, ns, 3, D]),
                        )
                        for s in range(ns):
                            nc.scalar.mul(
                                mv[:, s, HS:HD],
                                hps[:, s * HD + HS : s * HD + HD],
                                exf[:, s * H + 3 : s * H + 4],
                            )
                        # accumulate [sum(e*h) | sum(e)] over the window
                        for s in range(ns):
                            q = s0 + s
                            nc.tensor.matmul(
                                accdns[:],
                                lhsT=oh_all[:, (s0 + s) * 128 : (s0 + s + 1) * 128],
                                rhs=mv[:, s, :],
                                start=(q == 0),
                                stop=(q == kj - 1),
                            )
                    # finalize: out = acc / (dns + eps) + bias
                    dnse = fpool.tile([128, H], f32, tag="dnse")
                    nc.vector.tensor_scalar_add(dnse[:], accdns[:, HD : HD + H], EPS)
                    dnr = fpool.tile([128, H], f32, tag="dnr")
                    nc.vector.reciprocal(dnr[:], dnse[:])
                    outw = outb[:, k * HD : (k + 1) * HD]
                    nc.vector.tensor_mul(
                        outw.rearrange("p (h d) -> p h d", d=D),
                        accdns[:, 0:HD].rearrange("p (h d) -> p h d", d=D),
                        dnr[:].unsqueeze(-1).to_broadcast([128, H, D]),
                    )
                    nc.vector.tensor_add(outw, outw, bias_bc[:])
                    if k == gnb - 1:
                        nc.sync.dma_start(
                            out_d[g0 * 128 : (g0 + gnb) * 128, :].rearrange(
                                "(k p) c -> p k c", p=128
                            ),
                            outb[:, 0 : gnb * HD].rearrange("p (k c) -> p k c", c=HD),
                        )
    nc.compile()
    # compile()'s late passes (act-table loads, hostgen rebases) can leave
    # >1-wait instructions behind; one more split pass clears them (the TRN2
    # ISA allows a single sem wait per compute instruction).
    nc.generate_event_semaphores()
    return nc


def _stage_inputs(x, W, att_src, att_dst, bias, ilow, ihigh, oh_u, ohT_u):
    x = np.asarray(x, dtype=np.float32)
    x_il = _interleave_x(x)
    x_lo = np.ascontiguousarray(x_il[:SPLIT])
    x_hi = np.ascontiguousarray(x_il[SPLIT:])
    asrc_row = np.ascontiguousarray(np.asarray(att_src, np.float32).reshape(1, HD))
    adst_row = np.ascontiguousarray(np.asarray(att_dst, np.float32).reshape(1, HD))
    bias_row = np.ascontiguousarray(np.asarray(bias, np.float32).reshape(1, HD))
    in_maps = []
    for c in range(NCORES):
        xdT = np.zeros((IN_DIM, WROWS), dtype=np.float32)
        xdT[:, :NPC] = x[c * NPC : (c + 1) * NPC].T
        in_maps.append(
            {
                "x_lo": x_lo,
                "x_hi": x_hi,
                "xdstT": np.ascontiguousarray(xdT),
                "W": np.asarray(W, np.float32),
                "att_src": asrc_row,
                "att_dst": adst_row,
                "bias": bias_row,
                "ilow": np.ascontiguousarray(
                    ilow[c].transpose(1, 0, 2).reshape(128, -1)
                ),
                "ihigh": np.ascontiguousarray(
                    ihigh[c].transpose(1, 0, 2).reshape(128, -1)
                ),
                "oh": np.ascontiguousarray(oh_u[c]),
                "ohT": np.ascontiguousarray(ohT_u[c]),
            }
        )
    return in_maps


def kernel(x, edge_index, W, att_src, att_dst, bias):
    global LAST_RESULTS
    from concourse.bass_utils import run_bass_kernel_spmd

    edge_index = np.asarray(edge_index)
    ilow, ihigh, oh_u, ohT_u, KL, KH = _prep_host(edge_index)
    nc = _build_program(KL, KH)
    in_maps = _stage_inputs(x, W, att_src, att_dst, bias, ilow, ihigh, oh_u, ohT_u)

    res = run_bass_kernel_spmd(nc, in_maps, list(range(NCORES)))
    LAST_RESULTS = res

    out = np.empty((N, HD), dtype=np.float32)
    for c in range(NCORES):
        out[c * NPC : (c + 1) * NPC] = res.results[c]["out"][:NPC]
    return out
